# revision 32
# baseline (speedup 1.0000x reference)
"""DeepFM forward kernel for 8 Trainium2 NeuronCores (Bass/Tile), v6.

Single-phase design (structure found via ntff profiling):

  - Data-parallel over batch: B=16384 -> 2048 rows/core; tables+weights
    replicated.
  - Fields 0/1 (vocab 31360/6807): per-(field, j-tile) transposed SWDGE
    gathers from [size, 128] bf16 tables -> feature-major [128, 512]
    emb tiles.  Only 8 gathers/core: the partition-strided RX transfer
    (~4.5us per 512-row gather) is the head-latency wall, so fields 2/3
    (vocab 18/94) are computed WITHOUT gathers: the host sends a
    [94, 2, b_loc] fp8 one-hot encoding of their ids and the PE
    contracts it against host-premultiplied tables (tab_f @ W1_f), the
    exact same matmul count as the gathered path.
  - fc linear term: one whole-field indirect DMA per field (batch-major
    out); v3's 64 chunked indirect DMAs paid 64x the ~1us SWDGE fixed
    cost.
  - FM rowsum/rowsumsq: ones-vector matmuls over the emb tiles (f0/f1)
    plus per-row table-stat vectors contracted with the one-hot (f2/f3),
    accumulated in one PSUM chain; the global-scalar partial is
    AllGather'd across the 8 cores in-kernel (floor ~5us, hidden under
    the MLP) -- no second kernel launch.
  - MLP in fp8 DoubleRow (2x FLOPs via K=256/pass; a warm PE issues one
    512-col matmul every ~216ns).  PE emission is j-serial and gap-free;
    PSUM->SBUF drains split scalar(10)/DVE(6) per layer-mt.
  - Tail: ypre (from L4) is DRAM-bounce-transposed to batch-major
    [128,16], lin added, sigmoid with the AllGather'd S as bias.
"""

import os
import numpy as np
import ml_dtypes

# ---- problem constants (hardcoded; kernel.py must be self-contained) ----
TOTAL = 38279
CAT_SIZES = [31360, 6807, 18, 94]
EMB = 128
F = 4
B = 16384
N_CORES = 8
P = 128
NB = 512                       # matmul moving width (batch columns)
S23 = 94                       # one-hot partition count for fields 2/3
OFFSETS_NP = np.array([0, 31360, 38167, 38185], dtype=np.int32)

_build_cache = {}


def _build_main(b_loc, n_cores, use_cc):
    import concourse.bass as bass
    import concourse.mybir as mybir
    import concourse.tile as tile
    from concourse import bacc, library_config

    f32 = mybir.dt.float32
    bf16 = mybir.dt.bfloat16
    fp8 = mybir.dt.float8e4
    i16 = mybir.dt.int16
    i32 = mybir.dt.int32
    AF = mybir.ActivationFunctionType
    ALU = mybir.AluOpType
    AX = mybir.AxisListType
    DR = mybir.MatmulPerfMode.DoubleRow

    NJ = b_loc // NB             # 4 j-tiles
    NIX = NB // 16               # idx cols per (field, j) block
    NCH = b_loc // P             # 16 batch chunks of 128

    nc = bacc.Bacc(
        "TRN2",
        target_bir_lowering=False,
        debug=False,
        num_devices=n_cores,
    )

    # ---- DRAM I/O ----
    tabs = [
        nc.dram_tensor(f"tab{f}", [CAT_SIZES[f], EMB], bf16,
                       kind="ExternalInput").ap()
        for f in range(2)
    ]
    fc_d = nc.dram_tensor("fc", [TOTAL, 1], f32, kind="ExternalInput").ap()
    ix_d = nc.dram_tensor("ix", [P, NJ * 2 * NIX], i16,
                          kind="ExternalInput").ap()
    xig_d = nc.dram_tensor("xig", [P, F, NCH], i32, kind="ExternalInput").ap()
    oh_d = nc.dram_tensor("oh23", [S23, 2, b_loc], fp8,
                          kind="ExternalInput").ap()
    tw_d = nc.dram_tensor("tw23q", [S23, 2, 2048], fp8,
                          kind="ExternalInput").ap()
    sv23_d = nc.dram_tensor("svec23", [S23, 4], bf16,
                            kind="ExternalInput").ap()
    w1q_d = nc.dram_tensor("w1q", [P, 2, 2048], fp8, kind="ExternalInput").ap()
    w2q_d = nc.dram_tensor("w2q", [P, 16, 1024], fp8, kind="ExternalInput").ap()
    w3q_d = nc.dram_tensor("w3q", [P, 8, 512], fp8, kind="ExternalInput").ap()
    w4q_d = nc.dram_tensor("w4q", [P, 4], fp8, kind="ExternalInput").ap()
    b1p_d = nc.dram_tensor("b1p", [P, 16], f32, kind="ExternalInput").ap()
    b2p_d = nc.dram_tensor("b2p", [P, 8], f32, kind="ExternalInput").ap()
    b3p_d = nc.dram_tensor("b3p", [P, 4], f32, kind="ExternalInput").ap()
    bc_d = nc.dram_tensor("bconst", [1, 1], f32, kind="ExternalInput").ap()
    if use_cc:
        y_d = nc.dram_tensor("y", [b_loc, 1], f32, kind="ExternalOutput").ap()
    else:
        ylin_d = nc.dram_tensor("ylin", [P, NCH], f32,
                                kind="ExternalOutput").ap()
        gpart_d = nc.dram_tensor("gpart", [1, 1], f32,
                                 kind="ExternalOutput").ap()

    with tile.TileContext(nc) as tc:
        with (
            tc.tile_pool(name="const", bufs=1) as const,
            tc.tile_pool(name="gat", bufs=1) as gat,
            tc.tile_pool(name="act", bufs=2) as actp,
            tc.tile_pool(name="psmm", bufs=2, space="PSUM") as psum_mm,
            tc.tile_pool(name="psfm", bufs=1, space="PSUM") as psum_fm,
            tc.tile_pool(name="psl4", bufs=2, space="PSUM") as psum_l4,
            tc.tile_pool(name="dram", bufs=1, space="DRAM") as dram,
        ):
            nc.gpsimd.load_library(library_config.mlp)

            # ---- early-needed inputs on the scalar HWDGE queue (parallel
            # to the sync queue carrying the big weights) ----
            ix_sb = const.tile([P, NJ * 2 * NIX], i16, tag="ix_sb")
            nc.scalar.dma_start(ix_sb[:], ix_d)
            oh23 = const.tile([S23, 2, b_loc], fp8, tag="oh23")
            nc.scalar.dma_start(oh23[:], oh_d)
            tw23 = const.tile([S23, 2, 2048], fp8, tag="tw23")
            nc.scalar.dma_start(tw23[:], tw_d)
            xig = const.tile([P, F, NCH], i32, tag="xig")
            nc.scalar.dma_start(xig[:], xig_d)
            sv23 = const.tile([S23, 4], bf16, tag="sv23")
            nc.scalar.dma_start(sv23[:], sv23_d)
            # sync queue: L1 weights first, then the rest in need order
            w1q = const.tile([P, 2, 2048], fp8, tag="w1q")
            nc.sync.dma_start(w1q[:], w1q_d)
            bc_sb = const.tile([1, 1], f32, tag="bc_sb")
            nc.sync.dma_start(bc_sb[:], bc_d)
            b1p = const.tile([P, 16], f32, tag="b1p")
            nc.sync.dma_start(b1p[:], b1p_d)
            b2p = const.tile([P, 8], f32, tag="b2p")
            nc.sync.dma_start(b2p[:], b2p_d)
            b3p = const.tile([P, 4], f32, tag="b3p")
            nc.sync.dma_start(b3p[:], b3p_d)
            w4q = const.tile([P, 4], fp8, tag="w4q")
            nc.sync.dma_start(w4q[:], w4q_d)
            ones_col = const.tile([P, 1], bf16, tag="ones_col")
            nc.vector.memset(ones_col[:], 1.0)
            w2q = const.tile([P, 16, 1024], fp8, tag="w2q")
            nc.sync.dma_start(w2q[:], w2q_d)
            w3q = const.tile([P, 8, 512], fp8, tag="w3q")
            nc.sync.dma_start(w3q[:], w3q_d)

            ypre_sb = const.tile([1, b_loc], f32, tag="ypre_sb")
            ydram = dram.tile([1, b_loc], f32, tag="ydram")
            gacc = const.tile([1, NB], f32, tag="gacc")
            nc.vector.memset(gacc[:], 0.0)
            fcv = const.tile([P, F, NCH], f32, tag="fcv")

            def ixsl(f, j):
                k = (j * 2 + f) * NIX
                return ix_sb[:, k:k + NIX]

            # ---- gathers (fields 0/1 only), then fc indirects ----
            G = {}
            for j in range(NJ):
                for f in range(2):
                    g = gat.tile([P, 1, NB], bf16, tag=f"g{f}_{j}",
                                 name=f"g{f}_{j}")
                    nc.gpsimd.dma_gather(
                        g[:], tabs[f], ixsl(f, j), NB, NB, EMB,
                        transpose=True, single_packet=False,
                    )
                    G[(f, j)] = g
            for f in range(F):
                nc.gpsimd.indirect_dma_start(
                    out=fcv[:, f, :],
                    out_offset=None,
                    in_=fc_d,
                    in_offset=bass.IndirectOffsetOnAxis(ap=xig[:, f, :],
                                                        axis=0),
                )

            # bf16 copies of the one-hot (0/1 exact) for the FM stats chain
            OHB = {}
            for j in range(NJ):
                ohb = gat.tile([S23, 2, NB], bf16, tag=f"ohb{j}",
                               name=f"ohb{j}")
                jsl = slice(j * NB, (j + 1) * NB)
                nc.scalar.activation(ohb[:], oh23[:, :, jsl], AF.Copy)
                OHB[j] = ohb

            # fp8 pair tiles for L1 rhs: PT[j][:, c, :] = emb of field c
            PT = {}
            for j in range(NJ):
                PT[j] = gat.tile([P, 2, NB], fp8, tag=f"p{j}", name=f"p{j}")

            def emit_casts(j):
                for f in range(2):
                    nc.scalar.activation(PT[j][:, f, :], G[(f, j)][:, 0, :],
                                         AF.Copy)

            SQ = {}

            def emit_squares(j):
                for f in range(2):
                    sq = gat.tile([P, NB], bf16, tag=f"sq{f}_{j}",
                                  name=f"sq{f}_{j}")
                    nc.vector.tensor_tensor(out=sq[:], in0=G[(f, j)][:, 0, :],
                                            in1=G[(f, j)][:, 0, :],
                                            op=ALU.mult)
                    SQ[(f, j)] = sq

            def emit_fm_mm(j):
                # one-hot stats first: they don't depend on gather arrivals
                psA = psum_fm.tile([1, NB], f32, tag="psA", name=f"psA{j}")
                for c in range(2):
                    nc.tensor.matmul(psA[:], lhsT=sv23[:, c:c + 1],
                                     rhs=OHB[j][:, c, :], start=(c == 0),
                                     stop=False)
                for f in range(2):
                    nc.tensor.matmul(psA[:], lhsT=ones_col[:],
                                     rhs=G[(f, j)][:, 0, :], start=False,
                                     stop=(f == 1))
                psB = psum_fm.tile([1, NB], f32, tag="psB", name=f"psB{j}")
                for c in range(2):
                    nc.tensor.matmul(psB[:], lhsT=sv23[:, 2 + c:3 + c],
                                     rhs=OHB[j][:, c, :], start=(c == 0),
                                     stop=False)
                for f in range(2):
                    nc.tensor.matmul(psB[:], lhsT=ones_col[:],
                                     rhs=SQ[(f, j)][:], start=False,
                                     stop=(f == 1))
                return psA, psB

            def emit_fm_tail(j, psA, psB):
                rs = actp.tile([1, NB], f32, tag="fmr", name=f"fmr{j}")
                nc.scalar.activation(rs[:], psA[:], AF.Copy)
                t1 = actp.tile([1, NB], f32, tag="fmt", name=f"fmt{j}")
                nc.vector.tensor_tensor(out=t1[:], in0=rs[:], in1=rs[:],
                                        op=ALU.mult)
                nc.vector.tensor_tensor(out=t1[:], in0=t1[:], in1=psB[:],
                                        op=ALU.subtract)
                nc.vector.tensor_tensor(out=gacc[:], in0=gacc[:], in1=t1[:],
                                        op=ALU.add)

            def act_relu(on_scalar, dst, ps_slice, bias_ap):
                if on_scalar:
                    nc.scalar.activation(dst, ps_slice, AF.Relu, bias=bias_ap)
                else:
                    nc.vector.tensor_scalar(dst, ps_slice, bias_ap, 0.0,
                                            ALU.add, ALU.max)

            # ---- MLP layers for one j-tile ----
            H = {}

            def emit_l1(j):
                jsl = slice(j * NB, (j + 1) * NB)
                H1 = [actp.tile([P, 2, NB], fp8, tag=f"h1_{g}",
                                name=f"h1_{g}_{j}") for g in range(8)]
                H[(1, j)] = H1
                for mt in range(16):
                    q = mt % 2
                    if q == 0:
                        ps = psum_mm.tile([P, 2, NB], f32, tag="mm",
                                          name=f"mm1_{mt}_{j}")
                    nc.tensor.matmul(
                        ps[:, q, :], lhsT=w1q[:, :, mt * P:(mt + 1) * P],
                        rhs=PT[j][:], start=True, stop=False, perf_mode=DR)
                    nc.tensor.matmul(
                        ps[:, q, :], lhsT=tw23[:, :, mt * P:(mt + 1) * P],
                        rhs=oh23[:, :, jsl], start=False, stop=True,
                        perf_mode=DR)
                    # scalar is faster at PSUM drains: give it 10 of 16
                    act_relu(mt % 8 < 5, H1[mt // 2][:, mt % 2, :],
                             ps[:, q, :], b1p[:, mt:mt + 1])

            def emit_layer(j, lno, KG, MT, wq, bp, rhs_of):
                Hout = [actp.tile([P, 2, NB], fp8, tag=f"h{lno}_{g}",
                                  name=f"h{lno}_{g}_{j}")
                        for g in range(MT // 2)]
                H[(lno, j)] = Hout
                for mt in range(MT):
                    q = mt % 2
                    if q == 0:
                        ps = psum_mm.tile([P, 2, NB], f32, tag="mm",
                                          name=f"mm{lno}_{mt}_{j}")
                    for g in range(KG):
                        nc.tensor.matmul(
                            ps[:, q, :],
                            lhsT=wq[:, 2 * g:2 * g + 2, mt * P:(mt + 1) * P],
                            rhs=rhs_of(g),
                            start=(g == 0), stop=(g == KG - 1),
                            perf_mode=DR,
                        )
                    act_relu(mt % 2 == 0, Hout[mt // 2][:, mt % 2, :],
                             ps[:, q, :], bp[:, mt:mt + 1])

            def emit_l2(j):
                emit_layer(j, 2, 8, 8, w2q, b2p, lambda g: H[(1, j)][g][:])

            def emit_l3(j):
                emit_layer(j, 3, 4, 4, w3q, b3p, lambda g: H[(2, j)][g][:])

            def emit_l4(j):
                jsl = slice(j * NB, (j + 1) * NB)
                H3 = H[(3, j)]
                ps4 = psum_l4.tile([1, NB], f32, tag="l4", name=f"l4_{j}")
                for kt in range(4):
                    nc.tensor.matmul(
                        ps4[:], lhsT=w4q[:, kt:kt + 1],
                        rhs=H3[kt // 2][:, kt % 2, :],
                        start=(kt == 0), stop=(kt == 3),
                    )
                nc.scalar.activation(ypre_sb[:, jsl], ps4[:], AF.Identity)
                nc.sync.dma_start(ydram[:, jsl], ypre_sb[:, jsl])

            # ---- software-pipelined emission: j-serial PE stream ----
            emit_casts(0)
            emit_squares(0)
            fm0 = emit_fm_mm(0)
            emit_fm_tail(0, *fm0)
            emit_l1(0)
            emit_casts(1)
            emit_squares(1)
            emit_l2(0)
            fm1 = emit_fm_mm(1)
            emit_fm_tail(1, *fm1)
            emit_l3(0)
            emit_l4(0)
            emit_l1(1)
            emit_casts(2)
            emit_squares(2)
            emit_l2(1)
            fm2 = emit_fm_mm(2)
            emit_fm_tail(2, *fm2)
            emit_l3(1)
            emit_l4(1)
            emit_l1(2)
            emit_casts(3)
            emit_squares(3)
            emit_l2(2)
            fm3 = emit_fm_mm(3)
            emit_fm_tail(3, *fm3)
            emit_l3(2)
            emit_l4(2)
            emit_l1(3)

            # FM partial -> cross-core AllGather (hidden under the MLP)
            gp = const.tile([1, 1], f32, tag="gp")
            nc.vector.reduce_sum(out=gp[:], in_=gacc[:], axis=AX.X)
            sv128 = const.tile([P, 1], f32, tag="sv128")
            if use_cc:
                gin = dram.tile([1, 1], f32, tag="gin")
                gout = dram.tile([1, n_cores], f32, tag="gout",
                                 addr_space="Shared")
                nc.gpsimd.dma_start(gin[:], gp[:])
                nc.gpsimd.collective_compute(
                    "AllGather",
                    mybir.AluOpType.bypass,
                    replica_groups=[list(range(n_cores))],
                    ins=[gin.opt()],
                    outs=[gout.opt()],
                )
                gsb = const.tile([1, n_cores], f32, tag="gsb")
                nc.sync.dma_start(gsb[:], gout[:])
                gsum = const.tile([1, 1], f32, tag="gsum")
                nc.vector.reduce_sum(out=gsum[:], in_=gsb[:], axis=AX.X)
                # S = bias + b4 + 0.5 * sum(gparts)
                sv = const.tile([1, 1], f32, tag="sv")
                nc.scalar.activation(sv[:], gsum[:], AF.Identity,
                                     bias=bc_sb[0:1, 0:1], scale=0.5)
                nc.gpsimd.partition_broadcast(sv128[:], sv[:])

            emit_l2(3)
            emit_l3(3)
            emit_l4(3)

            # ---- tail: batch-major combine ----
            linT = const.tile([P, NCH], f32, tag="linT")
            nc.vector.tensor_tensor(out=linT[:], in0=fcv[:, 0, :],
                                    in1=fcv[:, 1, :], op=ALU.add)
            lin2 = const.tile([P, NCH], f32, tag="lin2")
            nc.vector.tensor_tensor(out=lin2[:], in0=fcv[:, 2, :],
                                    in1=fcv[:, 3, :], op=ALU.add)
            nc.vector.tensor_tensor(out=linT[:], in0=linT[:], in1=lin2[:],
                                    op=ALU.add)
            ypreT = const.tile([P, NCH], f32, tag="ypreT")
            nc.sync.dma_start(
                ypreT[:], ydram.rearrange("o (c p) -> p (c o)", p=P))
            nc.vector.tensor_tensor(out=ypreT[:], in0=ypreT[:], in1=linT[:],
                                    op=ALU.add)
            if use_cc:
                ysb = const.tile([P, NCH], f32, tag="ysb")
                nc.scalar.activation(ysb[:], ypreT[:], AF.Sigmoid,
                                     bias=sv128[:])
                nc.sync.dma_start(y_d.rearrange("(c p) o -> p (c o)", p=P),
                                  ysb[:])
            else:
                nc.sync.dma_start(ylin_d, ypreT[:])
                nc.sync.dma_start(gpart_d, gp[:])

    nc.compile()
    return nc


def _build_b(b_loc, n_cores):
    """Fallback phase B (no-collective mode): y = sigmoid(ylin + S)."""
    import concourse.mybir as mybir
    import concourse.tile as tile
    from concourse import bacc

    f32 = mybir.dt.float32
    AF = mybir.ActivationFunctionType
    NCH = b_loc // P

    nc = bacc.Bacc(
        "TRN2",
        target_bir_lowering=False,
        debug=False,
        num_devices=n_cores,
    )
    yin_d = nc.dram_tensor("yin", [P, NCH], f32, kind="ExternalInput").ap()
    sv_d = nc.dram_tensor("sv", [P, 1], f32, kind="ExternalInput").ap()
    y_d = nc.dram_tensor("y", [b_loc, 1], f32, kind="ExternalOutput").ap()

    with tile.TileContext(nc) as tc:
        with tc.tile_pool(name="const", bufs=1) as const:
            yin = const.tile([P, NCH], f32, tag="yin")
            nc.sync.dma_start(yin[:], yin_d)
            sv = const.tile([P, 1], f32, tag="sv")
            nc.sync.dma_start(sv[:], sv_d)
            ysb = const.tile([P, NCH], f32, tag="ysb")
            nc.scalar.activation(ysb[:], yin[:], AF.Sigmoid, bias=sv[:])
            nc.sync.dma_start(y_d.rearrange("(c p) o -> p (c o)", p=P), ysb[:])

    nc.compile()
    return nc


def _get_program(phase, b_loc, n_cores, use_cc=True):
    key = (phase, b_loc, n_cores, use_cc)
    if key not in _build_cache:
        _build_cache[key] = (
            _build_main(b_loc, n_cores, use_cc) if phase == "A"
            else _build_b(b_loc, n_cores)
        )
    return _build_cache[key]


def _wrap_idx(lin_idx):
    """[n] int -> [128, n//16] int16 dma_gather index tile (16-wrap,
    replicated for the 8 Q7 cores)."""
    n = lin_idx.shape[0]
    wrap = lin_idx.astype(np.int16).reshape(n // 16, 16).T  # [16, n//16]
    return np.ascontiguousarray(np.tile(wrap, (8, 1)))


def _prep_shared(inputs):
    """Host-side table/weight prep shared by all cores."""
    bf = ml_dtypes.bfloat16
    f8 = ml_dtypes.float8_e4m3
    emb16 = np.asarray(inputs["emb_table"], np.float32).astype(bf)  # [T,128]
    W1 = np.asarray(inputs["W1"], np.float32)

    sh = {}
    # reference quirk: embedding lookup uses RAW (un-offset) ids into the
    # full table, so every field's table is the FIRST size_f rows
    for f in range(2):
        sh[f"tab{f}"] = np.ascontiguousarray(emb16[:CAT_SIZES[f]])
    sh["fc"] = np.ascontiguousarray(np.asarray(inputs["fc"], np.float32))

    # fields 2/3: host-premultiplied tables + per-row stat vectors
    tab2 = emb16[:CAT_SIZES[2]].astype(np.float32)   # [18, 128]
    tab3 = emb16[:CAT_SIZES[3]].astype(np.float32)   # [94, 128]
    tw = np.zeros((S23, 2, 2048), np.float32)
    tw[:CAT_SIZES[2], 0] = tab2 @ W1[2 * EMB:3 * EMB]
    tw[:CAT_SIZES[3], 1] = tab3 @ W1[3 * EMB:4 * EMB]
    sh["tw23q"] = np.ascontiguousarray(tw.astype(f8))
    svec = np.zeros((S23, 4), np.float32)
    svec[:CAT_SIZES[2], 0] = tab2.sum(1)
    svec[:CAT_SIZES[3], 1] = tab3.sum(1)
    svec[:CAT_SIZES[2], 2] = (tab2 * tab2).sum(1)
    svec[:CAT_SIZES[3], 3] = (tab3 * tab3).sum(1)
    sh["svec23"] = np.ascontiguousarray(svec.astype(bf))

    def dr_pack(w, kgroups):
        K, M = w.shape
        w = np.asarray(w, np.float32).reshape(kgroups, 2, P, M)
        return np.ascontiguousarray(
            w.transpose(2, 0, 1, 3).reshape(P, 2 * kgroups, M).astype(f8)
        )

    sh["w1q"] = dr_pack(W1[:2 * EMB], 1)
    sh["w2q"] = dr_pack(np.asarray(inputs["W2"]), 8)
    sh["w3q"] = dr_pack(np.asarray(inputs["W3"]), 4)
    sh["w4q"] = np.ascontiguousarray(
        np.asarray(inputs["W4"], np.float32).reshape(4, P).T.astype(f8)
    )
    for name, mt in (("b1", 16), ("b2", 8), ("b3", 4)):
        sh[f"{name}p"] = np.ascontiguousarray(
            np.asarray(inputs[name], np.float32).reshape(mt, P).T
        )
    bconst = (np.asarray(inputs["bias"], np.float32).reshape(-1)[0]
              + np.asarray(inputs["b4"], np.float32).reshape(-1)[0])
    sh["bconst"] = np.full((1, 1), bconst, dtype=np.float32)
    return sh


def _pack_ix(xs):
    """Per-core [b_loc, 2] raw ids (fields 0/1) -> [128, NJ*2*NIX] int16."""
    b_loc = xs.shape[0]
    NJ = b_loc // NB
    cols = []
    for j in range(NJ):
        for f in range(2):
            cols.append(_wrap_idx(xs[j * NB:(j + 1) * NB, f]))
    return np.ascontiguousarray(np.concatenate(cols, axis=1))


def kernel(**inputs) -> np.ndarray:
    from concourse.bass_utils import run_bass_kernel_spmd

    n_cores = N_CORES
    b_loc = B // n_cores
    NCH = b_loc // P
    cores = list(range(n_cores))
    trace = bool(int(os.environ.get("KERNEL_TRACE", "0")))
    use_cc = not bool(int(os.environ.get("KERNEL_NO_CC", "0")))

    x_int = np.asarray(inputs["x"], np.float32).astype(np.int32)  # [B, F]
    shared = _prep_shared(inputs)
    f8 = ml_dtypes.float8_e4m3

    ncA = _get_program("A", b_loc, n_cores, use_cc)
    in_maps = []
    for c in range(n_cores):
        m = dict(shared)
        xs = x_int[c * b_loc:(c + 1) * b_loc]
        m["ix"] = _pack_ix(xs)
        m["xig"] = np.ascontiguousarray(
            (xs + OFFSETS_NP).reshape(NCH, P, F).transpose(1, 2, 0)
        )
        oh = np.zeros((S23, 2, b_loc), np.float32)
        oh[xs[:, 2], 0, np.arange(b_loc)] = 1.0
        oh[xs[:, 3], 1, np.arange(b_loc)] = 1.0
        m["oh23"] = np.ascontiguousarray(oh.astype(f8))
        in_maps.append(m)
    resA = run_bass_kernel_spmd(ncA, in_maps, core_ids=cores, trace=trace)

    if use_cc:
        kernel._last_results = (resA,)
        kernel._last_exec_ns = resA.exec_time_ns
        kernel._last_exec_parts = (resA.exec_time_ns,)
        out = np.concatenate(
            [np.asarray(r["y"], np.float32).reshape(b_loc) for r in resA.results]
        )
        return out.reshape(B, 1).astype(np.float32)

    # ---- fallback: host-side reduction + tiny phase B ----
    g = np.float32(0.0)
    for r in resA.results:
        g = np.float32(g + np.float32(r["gpart"][0, 0]))
    S = np.float32(shared["bconst"][0, 0] + 0.5 * g)

    ncB = _get_program("B", b_loc, n_cores)
    sv = np.full((P, 1), S, dtype=np.float32)
    in_maps_b = [
        {"yin": np.asarray(resA.results[c]["ylin"], np.float32), "sv": sv}
        for c in range(n_cores)
    ]
    resB = run_bass_kernel_spmd(ncB, in_maps_b, core_ids=cores, trace=trace)

    kernel._last_results = (resA, resB)
    a_ns, b_ns = resA.exec_time_ns, resB.exec_time_ns
    kernel._last_exec_ns = (
        (a_ns or 0) + (b_ns or 0) if (a_ns is not None or b_ns is not None)
        else None
    )
    kernel._last_exec_parts = (a_ns, b_ns)
    out = np.concatenate([r["y"] for r in resB.results], axis=0)
    return out.astype(np.float32)


# revision 37
# speedup vs baseline: 1.0741x; 1.0741x over previous
"""DeepFM forward kernel for 8 Trainium2 NeuronCores (Bass/Tile), v6.

Single-phase design (structure found via ntff profiling):

  - Data-parallel over batch: B=16384 -> 2048 rows/core; tables+weights
    replicated.
  - Fields 0/1 (vocab 31360/6807): per-(field, j-tile) transposed SWDGE
    gathers from [size, 128] bf16 tables -> feature-major [128, 512]
    emb tiles.  Only 8 gathers/core: the partition-strided RX transfer
    (~4.5us per 512-row gather) is the head-latency wall, so fields 2/3
    (vocab 18/94) are computed WITHOUT gathers: the host sends a
    [94, 2, b_loc] fp8 one-hot encoding of their ids and the PE
    contracts it against host-premultiplied tables (tab_f @ W1_f), the
    exact same matmul count as the gathered path.
  - fc linear term: one whole-field indirect DMA per field (batch-major
    out); v3's 64 chunked indirect DMAs paid 64x the ~1us SWDGE fixed
    cost.
  - FM rowsum/rowsumsq: ones-vector matmuls over the emb tiles (f0/f1)
    plus per-row table-stat vectors contracted with the one-hot (f2/f3),
    accumulated in one PSUM chain; the global-scalar partial is
    AllGather'd across the 8 cores in-kernel (floor ~5us, hidden under
    the MLP) -- no second kernel launch.
  - MLP in fp8 DoubleRow (2x FLOPs via K=256/pass; a warm PE issues one
    512-col matmul every ~216ns).  PE emission is j-serial and gap-free;
    PSUM->SBUF drains split scalar(10)/DVE(6) per layer-mt.
  - Tail: ypre (from L4) is DRAM-bounce-transposed to batch-major
    [128,16], lin added, sigmoid with the AllGather'd S as bias.
"""

import os
import numpy as np
import ml_dtypes

# ---- problem constants (hardcoded; kernel.py must be self-contained) ----
TOTAL = 38279
CAT_SIZES = [31360, 6807, 18, 94]
EMB = 128
F = 4
B = 16384
N_CORES = 8
P = 128
NB = 512                       # matmul moving width (batch columns)
S23 = 94                       # one-hot partition count for fields 2/3
OFFSETS_NP = np.array([0, 31360, 38167, 38185], dtype=np.int32)

_build_cache = {}


def _build_main(b_loc, n_cores, use_cc):
    import concourse.bass as bass
    import concourse.mybir as mybir
    import concourse.tile as tile
    from concourse import bacc, library_config

    f32 = mybir.dt.float32
    bf16 = mybir.dt.bfloat16
    fp8 = mybir.dt.float8e4
    i16 = mybir.dt.int16
    i32 = mybir.dt.int32
    AF = mybir.ActivationFunctionType
    ALU = mybir.AluOpType
    AX = mybir.AxisListType
    DR = mybir.MatmulPerfMode.DoubleRow

    NJ = b_loc // NB             # 4 j-tiles
    NIX = NB // 16               # idx cols per (field, j) block
    NCH = b_loc // P             # 16 batch chunks of 128

    nc = bacc.Bacc(
        "TRN2",
        target_bir_lowering=False,
        debug=False,
        num_devices=n_cores,
    )

    # ---- DRAM I/O ----
    tabs = [
        nc.dram_tensor(f"tab{f}", [CAT_SIZES[f], EMB], bf16,
                       kind="ExternalInput").ap()
        for f in range(2)
    ]
    fc_d = nc.dram_tensor("fc", [TOTAL, 1], f32, kind="ExternalInput").ap()
    ix_d = nc.dram_tensor("ix", [P, NJ * 2 * NIX], i16,
                          kind="ExternalInput").ap()
    xig_d = nc.dram_tensor("xig", [P, F, NCH], i32, kind="ExternalInput").ap()
    oh_d = nc.dram_tensor("oh23", [S23, 2, b_loc], fp8,
                          kind="ExternalInput").ap()
    tw_d = nc.dram_tensor("tw23q", [S23, 2, 2048], fp8,
                          kind="ExternalInput").ap()
    sv23_d = nc.dram_tensor("svec23", [S23, 4], bf16,
                            kind="ExternalInput").ap()
    w1q_d = nc.dram_tensor("w1q", [P, 2, 2048], fp8, kind="ExternalInput").ap()
    w2q_d = nc.dram_tensor("w2q", [P, 16, 1024], fp8, kind="ExternalInput").ap()
    w3q_d = nc.dram_tensor("w3q", [P, 8, 512], fp8, kind="ExternalInput").ap()
    w4q_d = nc.dram_tensor("w4q", [P, 4], fp8, kind="ExternalInput").ap()
    b1p_d = nc.dram_tensor("b1p", [P, 16], f32, kind="ExternalInput").ap()
    b2p_d = nc.dram_tensor("b2p", [P, 8], f32, kind="ExternalInput").ap()
    b3p_d = nc.dram_tensor("b3p", [P, 4], f32, kind="ExternalInput").ap()
    bc_d = nc.dram_tensor("bconst", [1, 1], f32, kind="ExternalInput").ap()
    if use_cc:
        y_d = nc.dram_tensor("y", [b_loc, 1], f32, kind="ExternalOutput").ap()
    else:
        ylin_d = nc.dram_tensor("ylin", [P, NCH], f32,
                                kind="ExternalOutput").ap()
        gpart_d = nc.dram_tensor("gpart", [1, 1], f32,
                                 kind="ExternalOutput").ap()

    with tile.TileContext(nc) as tc:
        with (
            tc.tile_pool(name="const", bufs=1) as const,
            tc.tile_pool(name="gat", bufs=1) as gat,
            tc.tile_pool(name="act", bufs=2) as actp,
            tc.tile_pool(name="psmm", bufs=2, space="PSUM") as psum_mm,
            tc.tile_pool(name="psfm", bufs=1, space="PSUM") as psum_fm,
            tc.tile_pool(name="psl4", bufs=2, space="PSUM") as psum_l4,
            tc.tile_pool(name="dram", bufs=1, space="DRAM") as dram,
        ):
            nc.gpsimd.load_library(library_config.mlp)

            # ---- early-needed inputs on the scalar HWDGE queue (parallel
            # to the sync queue carrying the big weights) ----
            ix_sb = const.tile([P, NJ * 2 * NIX], i16, tag="ix_sb")
            nc.scalar.dma_start(ix_sb[:], ix_d)
            oh23 = const.tile([S23, 2, b_loc], fp8, tag="oh23")
            nc.scalar.dma_start(oh23[:], oh_d)
            tw23 = const.tile([S23, 2, 2048], fp8, tag="tw23")
            nc.scalar.dma_start(tw23[:], tw_d)
            xig = const.tile([P, F, NCH], i32, tag="xig")
            nc.scalar.dma_start(xig[:], xig_d)
            sv23 = const.tile([S23, 4], bf16, tag="sv23")
            nc.scalar.dma_start(sv23[:], sv23_d)
            # sync queue: L1 weights first, then the rest in need order
            w1q = const.tile([P, 2, 2048], fp8, tag="w1q")
            nc.sync.dma_start(w1q[:], w1q_d)
            bc_sb = const.tile([1, 1], f32, tag="bc_sb")
            nc.sync.dma_start(bc_sb[:], bc_d)
            b1p = const.tile([P, 16], f32, tag="b1p")
            nc.sync.dma_start(b1p[:], b1p_d)
            b2p = const.tile([P, 8], f32, tag="b2p")
            nc.sync.dma_start(b2p[:], b2p_d)
            b3p = const.tile([P, 4], f32, tag="b3p")
            nc.sync.dma_start(b3p[:], b3p_d)
            w4q = const.tile([P, 4], fp8, tag="w4q")
            nc.sync.dma_start(w4q[:], w4q_d)
            ones_col = const.tile([P, 1], bf16, tag="ones_col")
            nc.vector.memset(ones_col[:], 1.0)
            w2q = const.tile([P, 16, 1024], fp8, tag="w2q")
            nc.sync.dma_start(w2q[:], w2q_d)
            w3q = const.tile([P, 8, 512], fp8, tag="w3q")
            nc.sync.dma_start(w3q[:], w3q_d)

            ypre_sb = const.tile([1, b_loc], f32, tag="ypre_sb")
            ydram = dram.tile([1, b_loc], f32, tag="ydram")
            gacc = const.tile([1, NB], f32, tag="gacc")
            nc.vector.memset(gacc[:], 0.0)
            fcv = const.tile([P, F, NCH], f32, tag="fcv")

            def ixsl(f, j):
                k = (j * 2 + f) * NIX
                return ix_sb[:, k:k + NIX]

            # ---- gathers (fields 0/1 only), then fc indirects ----
            G = {}
            for j in range(NJ):
                for f in range(2):
                    g = gat.tile([P, 1, NB], bf16, tag=f"g{f}_{j}",
                                 name=f"g{f}_{j}")
                    nc.gpsimd.dma_gather(
                        g[:], tabs[f], ixsl(f, j), NB, NB, EMB,
                        transpose=True, single_packet=False,
                    )
                    G[(f, j)] = g
            for f in range(F):
                nc.gpsimd.indirect_dma_start(
                    out=fcv[:, f, :],
                    out_offset=None,
                    in_=fc_d,
                    in_offset=bass.IndirectOffsetOnAxis(ap=xig[:, f, :],
                                                        axis=0),
                )

            # bf16 copies of the one-hot (0/1 exact) for the FM stats chain
            OHB = {}
            for j in range(NJ):
                ohb = gat.tile([S23, 2, NB], bf16, tag=f"ohb{j}",
                               name=f"ohb{j}")
                jsl = slice(j * NB, (j + 1) * NB)
                nc.scalar.activation(ohb[:], oh23[:, :, jsl], AF.Copy)
                OHB[j] = ohb

            # fp8 pair tiles for L1 rhs: PT[j][:, c, :] = emb of field c
            PT = {}
            for j in range(NJ):
                PT[j] = gat.tile([P, 2, NB], fp8, tag=f"p{j}", name=f"p{j}")

            def emit_casts(j):
                for f in range(2):
                    nc.scalar.activation(PT[j][:, f, :], G[(f, j)][:, 0, :],
                                         AF.Copy)

            SQ = {}

            def emit_squares(j):
                for f in range(2):
                    sq = gat.tile([P, NB], bf16, tag=f"sq{f}_{j}",
                                  name=f"sq{f}_{j}")
                    nc.vector.tensor_tensor(out=sq[:], in0=G[(f, j)][:, 0, :],
                                            in1=G[(f, j)][:, 0, :],
                                            op=ALU.mult)
                    SQ[(f, j)] = sq

            def emit_fm_mm(j):
                # one-hot stats first: they don't depend on gather arrivals
                psA = psum_fm.tile([1, NB], f32, tag="psA", name=f"psA{j}")[:]
                psB = psum_fm.tile([1, NB], f32, tag="psB", name=f"psB{j}")[:]
                for c in range(2):
                    nc.tensor.matmul(psA, lhsT=sv23[:, c:c + 1],
                                     rhs=OHB[j][:, c, :], start=(c == 0),
                                     stop=False)
                for f in range(2):
                    nc.tensor.matmul(psA, lhsT=ones_col[:],
                                     rhs=G[(f, j)][:, 0, :], start=False,
                                     stop=(f == 1))
                for c in range(2):
                    nc.tensor.matmul(psB, lhsT=sv23[:, 2 + c:3 + c],
                                     rhs=OHB[j][:, c, :], start=(c == 0),
                                     stop=False)
                for f in range(2):
                    nc.tensor.matmul(psB, lhsT=ones_col[:],
                                     rhs=SQ[(f, j)][:], start=False,
                                     stop=(f == 1))
                return psA, psB

            def emit_fm_tail(j, psA, psB):
                rs = actp.tile([1, NB], f32, tag="fmr", name=f"fmr{j}")
                nc.scalar.activation(rs[:], psA, AF.Copy)
                t1 = actp.tile([1, NB], f32, tag="fmt", name=f"fmt{j}")
                nc.vector.tensor_tensor(out=t1[:], in0=rs[:], in1=rs[:],
                                        op=ALU.mult)
                nc.vector.tensor_tensor(out=t1[:], in0=t1[:], in1=psB,
                                        op=ALU.subtract)
                nc.vector.tensor_tensor(out=gacc[:], in0=gacc[:], in1=t1[:],
                                        op=ALU.add)

            def act_relu(on_scalar, dst, ps_slice, bias_ap):
                if on_scalar:
                    nc.scalar.activation(dst, ps_slice, AF.Relu, bias=bias_ap)
                else:
                    nc.vector.tensor_scalar(dst, ps_slice, bias_ap, 0.0,
                                            ALU.add, ALU.max)

            # ---- MLP layers for one j-tile ----
            H = {}

            def emit_l1(j):
                jsl = slice(j * NB, (j + 1) * NB)
                H1 = [actp.tile([P, 2, NB], fp8, tag=f"h1_{g}",
                                name=f"h1_{g}_{j}") for g in range(8)]
                H[(1, j)] = H1
                for mt in range(16):
                    q = mt % 2
                    if q == 0:
                        ps = psum_mm.tile([P, 2, NB], f32, tag="mm",
                                          name=f"mm1_{mt}_{j}")
                    nc.tensor.matmul(
                        ps[:, q, :], lhsT=w1q[:, :, mt * P:(mt + 1) * P],
                        rhs=PT[j][:], start=True, stop=False, perf_mode=DR)
                    nc.tensor.matmul(
                        ps[:, q, :], lhsT=tw23[:, :, mt * P:(mt + 1) * P],
                        rhs=oh23[:, :, jsl], start=False, stop=True,
                        perf_mode=DR)
                    # scalar is faster at PSUM drains: give it 10 of 16
                    act_relu(mt % 8 < 5, H1[mt // 2][:, mt % 2, :],
                             ps[:, q, :], b1p[:, mt:mt + 1])

            def emit_layer(j, lno, KG, MT, wq, bp, rhs_of):
                Hout = [actp.tile([P, 2, NB], fp8, tag=f"h{lno}_{g}",
                                  name=f"h{lno}_{g}_{j}")
                        for g in range(MT // 2)]
                H[(lno, j)] = Hout
                for mt in range(MT):
                    q = mt % 2
                    if q == 0:
                        ps = psum_mm.tile([P, 2, NB], f32, tag="mm",
                                          name=f"mm{lno}_{mt}_{j}")
                    for g in range(KG):
                        nc.tensor.matmul(
                            ps[:, q, :],
                            lhsT=wq[:, 2 * g:2 * g + 2, mt * P:(mt + 1) * P],
                            rhs=rhs_of(g),
                            start=(g == 0), stop=(g == KG - 1),
                            perf_mode=DR,
                        )
                    act_relu(mt % 2 == 0, Hout[mt // 2][:, mt % 2, :],
                             ps[:, q, :], bp[:, mt:mt + 1])

            def emit_l2(j):
                emit_layer(j, 2, 8, 8, w2q, b2p, lambda g: H[(1, j)][g][:])

            def emit_l3(j):
                emit_layer(j, 3, 4, 4, w3q, b3p, lambda g: H[(2, j)][g][:])

            def emit_l4(j):
                jsl = slice(j * NB, (j + 1) * NB)
                H3 = H[(3, j)]
                ps4 = psum_l4.tile([1, NB], f32, tag="l4", name=f"l4_{j}")
                for kt in range(4):
                    nc.tensor.matmul(
                        ps4[:], lhsT=w4q[:, kt:kt + 1],
                        rhs=H3[kt // 2][:, kt % 2, :],
                        start=(kt == 0), stop=(kt == 3),
                    )
                nc.scalar.activation(ypre_sb[:, jsl], ps4[:], AF.Identity)
                nc.sync.dma_start(ydram[:, jsl], ypre_sb[:, jsl])

            # ---- software-pipelined emission: j-serial PE stream ----
            emit_casts(0)
            emit_squares(0)
            fm0 = emit_fm_mm(0)
            emit_fm_tail(0, *fm0)
            emit_l1(0)
            emit_casts(1)
            emit_squares(1)
            emit_l2(0)
            fm1 = emit_fm_mm(1)
            emit_fm_tail(1, *fm1)
            emit_l3(0)
            emit_l4(0)
            emit_l1(1)
            emit_casts(2)
            emit_squares(2)
            emit_l2(1)
            fm2 = emit_fm_mm(2)
            emit_fm_tail(2, *fm2)
            emit_l3(1)
            emit_l4(1)
            emit_l1(2)
            emit_casts(3)
            emit_squares(3)
            emit_l2(2)
            fm3 = emit_fm_mm(3)
            emit_fm_tail(3, *fm3)
            emit_l3(2)
            emit_l4(2)
            emit_l1(3)

            # FM partial -> cross-core AllGather, triggered early on gpsimd
            # (the result is only consumed in the tail block below)
            gp = const.tile([1, 1], f32, tag="gp")
            nc.vector.reduce_sum(out=gp[:], in_=gacc[:], axis=AX.X)
            sv128 = const.tile([P, 1], f32, tag="sv128")
            if use_cc:
                gin = dram.tile([1, 1], f32, tag="gin")
                gout = dram.tile([1, n_cores], f32, tag="gout",
                                 addr_space="Shared")
                nc.gpsimd.dma_start(gin[:], gp[:])
                nc.gpsimd.collective_compute(
                    "AllGather",
                    mybir.AluOpType.bypass,
                    replica_groups=[list(range(n_cores))],
                    ins=[gin.opt()],
                    outs=[gout.opt()],
                )

            emit_l2(3)
            emit_l3(3)
            emit_l4(3)

            # ---- tail: batch-major combine ----
            # tile_wait_until keeps the scheduler from hoisting these ahead
            # of MLP drains on the same engines (their input DMAs complete
            # late; a hoisted wait would bubble the whole engine queue)
            with tc.tile_wait_until(0.15):
                if use_cc:
                    gsb = const.tile([1, n_cores], f32, tag="gsb")
                    nc.sync.dma_start(gsb[:], gout[:])
                    gsum = const.tile([1, 1], f32, tag="gsum")
                    nc.vector.reduce_sum(out=gsum[:], in_=gsb[:], axis=AX.X)
                    # S = bias + b4 + 0.5 * sum(gparts)
                    sv = const.tile([1, 1], f32, tag="sv")
                    nc.scalar.activation(sv[:], gsum[:], AF.Identity,
                                         bias=bc_sb[0:1, 0:1], scale=0.5)
                    nc.gpsimd.partition_broadcast(sv128[:], sv[:])
                linT = const.tile([P, NCH], f32, tag="linT")
                nc.vector.tensor_tensor(out=linT[:], in0=fcv[:, 0, :],
                                        in1=fcv[:, 1, :], op=ALU.add)
                lin2 = const.tile([P, NCH], f32, tag="lin2")
                nc.vector.tensor_tensor(out=lin2[:], in0=fcv[:, 2, :],
                                        in1=fcv[:, 3, :], op=ALU.add)
                nc.vector.tensor_tensor(out=linT[:], in0=linT[:],
                                        in1=lin2[:], op=ALU.add)
                ypreT = const.tile([P, NCH], f32, tag="ypreT")
                nc.sync.dma_start(
                    ypreT[:], ydram.rearrange("o (c p) -> p (c o)", p=P))
                nc.vector.tensor_tensor(out=ypreT[:], in0=ypreT[:],
                                        in1=linT[:], op=ALU.add)
                if use_cc:
                    ysb = const.tile([P, NCH], f32, tag="ysb")
                    nc.scalar.activation(ysb[:], ypreT[:], AF.Sigmoid,
                                         bias=sv128[:])
                    nc.sync.dma_start(
                        y_d.rearrange("(c p) o -> p (c o)", p=P), ysb[:])
                else:
                    nc.sync.dma_start(ylin_d, ypreT[:])
                    nc.sync.dma_start(gpart_d, gp[:])

    nc.compile()
    return nc


def _build_b(b_loc, n_cores):
    """Fallback phase B (no-collective mode): y = sigmoid(ylin + S)."""
    import concourse.mybir as mybir
    import concourse.tile as tile
    from concourse import bacc

    f32 = mybir.dt.float32
    AF = mybir.ActivationFunctionType
    NCH = b_loc // P

    nc = bacc.Bacc(
        "TRN2",
        target_bir_lowering=False,
        debug=False,
        num_devices=n_cores,
    )
    yin_d = nc.dram_tensor("yin", [P, NCH], f32, kind="ExternalInput").ap()
    sv_d = nc.dram_tensor("sv", [P, 1], f32, kind="ExternalInput").ap()
    y_d = nc.dram_tensor("y", [b_loc, 1], f32, kind="ExternalOutput").ap()

    with tile.TileContext(nc) as tc:
        with tc.tile_pool(name="const", bufs=1) as const:
            yin = const.tile([P, NCH], f32, tag="yin")
            nc.sync.dma_start(yin[:], yin_d)
            sv = const.tile([P, 1], f32, tag="sv")
            nc.sync.dma_start(sv[:], sv_d)
            ysb = const.tile([P, NCH], f32, tag="ysb")
            nc.scalar.activation(ysb[:], yin[:], AF.Sigmoid, bias=sv[:])
            nc.sync.dma_start(y_d.rearrange("(c p) o -> p (c o)", p=P), ysb[:])

    nc.compile()
    return nc


def _get_program(phase, b_loc, n_cores, use_cc=True):
    key = (phase, b_loc, n_cores, use_cc)
    if key not in _build_cache:
        _build_cache[key] = (
            _build_main(b_loc, n_cores, use_cc) if phase == "A"
            else _build_b(b_loc, n_cores)
        )
    return _build_cache[key]


def _wrap_idx(lin_idx):
    """[n] int -> [128, n//16] int16 dma_gather index tile (16-wrap,
    replicated for the 8 Q7 cores)."""
    n = lin_idx.shape[0]
    wrap = lin_idx.astype(np.int16).reshape(n // 16, 16).T  # [16, n//16]
    return np.ascontiguousarray(np.tile(wrap, (8, 1)))


def _prep_shared(inputs):
    """Host-side table/weight prep shared by all cores."""
    bf = ml_dtypes.bfloat16
    f8 = ml_dtypes.float8_e4m3
    emb16 = np.asarray(inputs["emb_table"], np.float32).astype(bf)  # [T,128]
    W1 = np.asarray(inputs["W1"], np.float32)

    sh = {}
    # reference quirk: embedding lookup uses RAW (un-offset) ids into the
    # full table, so every field's table is the FIRST size_f rows
    for f in range(2):
        sh[f"tab{f}"] = np.ascontiguousarray(emb16[:CAT_SIZES[f]])
    sh["fc"] = np.ascontiguousarray(np.asarray(inputs["fc"], np.float32))

    # fields 2/3: host-premultiplied tables + per-row stat vectors
    tab2 = emb16[:CAT_SIZES[2]].astype(np.float32)   # [18, 128]
    tab3 = emb16[:CAT_SIZES[3]].astype(np.float32)   # [94, 128]
    tw = np.zeros((S23, 2, 2048), np.float32)
    tw[:CAT_SIZES[2], 0] = tab2 @ W1[2 * EMB:3 * EMB]
    tw[:CAT_SIZES[3], 1] = tab3 @ W1[3 * EMB:4 * EMB]
    sh["tw23q"] = np.ascontiguousarray(tw.astype(f8))
    svec = np.zeros((S23, 4), np.float32)
    svec[:CAT_SIZES[2], 0] = tab2.sum(1)
    svec[:CAT_SIZES[3], 1] = tab3.sum(1)
    svec[:CAT_SIZES[2], 2] = (tab2 * tab2).sum(1)
    svec[:CAT_SIZES[3], 3] = (tab3 * tab3).sum(1)
    sh["svec23"] = np.ascontiguousarray(svec.astype(bf))

    def dr_pack(w, kgroups):
        K, M = w.shape
        w = np.asarray(w, np.float32).reshape(kgroups, 2, P, M)
        return np.ascontiguousarray(
            w.transpose(2, 0, 1, 3).reshape(P, 2 * kgroups, M).astype(f8)
        )

    sh["w1q"] = dr_pack(W1[:2 * EMB], 1)
    sh["w2q"] = dr_pack(np.asarray(inputs["W2"]), 8)
    sh["w3q"] = dr_pack(np.asarray(inputs["W3"]), 4)
    sh["w4q"] = np.ascontiguousarray(
        np.asarray(inputs["W4"], np.float32).reshape(4, P).T.astype(f8)
    )
    for name, mt in (("b1", 16), ("b2", 8), ("b3", 4)):
        sh[f"{name}p"] = np.ascontiguousarray(
            np.asarray(inputs[name], np.float32).reshape(mt, P).T
        )
    bconst = (np.asarray(inputs["bias"], np.float32).reshape(-1)[0]
              + np.asarray(inputs["b4"], np.float32).reshape(-1)[0])
    sh["bconst"] = np.full((1, 1), bconst, dtype=np.float32)
    return sh


def _pack_ix(xs):
    """Per-core [b_loc, 2] raw ids (fields 0/1) -> [128, NJ*2*NIX] int16."""
    b_loc = xs.shape[0]
    NJ = b_loc // NB
    cols = []
    for j in range(NJ):
        for f in range(2):
            cols.append(_wrap_idx(xs[j * NB:(j + 1) * NB, f]))
    return np.ascontiguousarray(np.concatenate(cols, axis=1))


def kernel(**inputs) -> np.ndarray:
    from concourse.bass_utils import run_bass_kernel_spmd

    n_cores = N_CORES
    b_loc = B // n_cores
    NCH = b_loc // P
    cores = list(range(n_cores))
    trace = bool(int(os.environ.get("KERNEL_TRACE", "0")))
    use_cc = not bool(int(os.environ.get("KERNEL_NO_CC", "0")))

    x_int = np.asarray(inputs["x"], np.float32).astype(np.int32)  # [B, F]
    shared = _prep_shared(inputs)
    f8 = ml_dtypes.float8_e4m3

    ncA = _get_program("A", b_loc, n_cores, use_cc)
    in_maps = []
    for c in range(n_cores):
        m = dict(shared)
        xs = x_int[c * b_loc:(c + 1) * b_loc]
        m["ix"] = _pack_ix(xs)
        m["xig"] = np.ascontiguousarray(
            (xs + OFFSETS_NP).reshape(NCH, P, F).transpose(1, 2, 0)
        )
        oh = np.zeros((S23, 2, b_loc), np.float32)
        oh[xs[:, 2], 0, np.arange(b_loc)] = 1.0
        oh[xs[:, 3], 1, np.arange(b_loc)] = 1.0
        m["oh23"] = np.ascontiguousarray(oh.astype(f8))
        in_maps.append(m)
    resA = run_bass_kernel_spmd(ncA, in_maps, core_ids=cores, trace=trace)

    if use_cc:
        kernel._last_results = (resA,)
        kernel._last_exec_ns = resA.exec_time_ns
        kernel._last_exec_parts = (resA.exec_time_ns,)
        out = np.concatenate(
            [np.asarray(r["y"], np.float32).reshape(b_loc) for r in resA.results]
        )
        return out.reshape(B, 1).astype(np.float32)

    # ---- fallback: host-side reduction + tiny phase B ----
    g = np.float32(0.0)
    for r in resA.results:
        g = np.float32(g + np.float32(r["gpart"][0, 0]))
    S = np.float32(shared["bconst"][0, 0] + 0.5 * g)

    ncB = _get_program("B", b_loc, n_cores)
    sv = np.full((P, 1), S, dtype=np.float32)
    in_maps_b = [
        {"yin": np.asarray(resA.results[c]["ylin"], np.float32), "sv": sv}
        for c in range(n_cores)
    ]
    resB = run_bass_kernel_spmd(ncB, in_maps_b, core_ids=cores, trace=trace)

    kernel._last_results = (resA, resB)
    a_ns, b_ns = resA.exec_time_ns, resB.exec_time_ns
    kernel._last_exec_ns = (
        (a_ns or 0) + (b_ns or 0) if (a_ns is not None or b_ns is not None)
        else None
    )
    kernel._last_exec_parts = (a_ns, b_ns)
    out = np.concatenate([r["y"] for r in resB.results], axis=0)
    return out.astype(np.float32)
